# revision 51
# baseline (speedup 1.0000x reference)
"""Paged GQA attention (sparse_attention nn_Attention_29867202576782) on 8 trn2 cores.

Data-parallel over the B=16 sequences (2 per core); 16 (seq, kv-head) pairs per
core, each attending 128 query-columns (SQ*G) over S_TOTAL=2048 keys. K and V
ship as fp8e3 (e3m4) slabs - the PE supports mixed-dtype matmuls (fp8 stationary
x fp16 moving and vice versa), which halves the KV DMA traffic (rel err 1.76e-2
vs the 2e-2 gate, measured end-to-end on HW with the real inputs).

With fp8 the kernel is ACT-bound (one exp pass over all 4.2M scores/core at
1 elem/lane/cycle), so everything is organized around a gapless Activation
stream:

- The 256-chunk (pair, s-chunk) stream is tiled into [128, 12, 128] fp32 PSUM
  score tiles (3 banks x 2 bufs) that deliberately cross pair boundaries: one
  exp instruction per 12-chunk tile amortizes the ~185ns fixed ACT access cost
  (26 exps instead of 33+). The first/last tiles are split into 2/2/4/4-chunk
  exps to start the stream early and retire it cheaply.
- o and l accumulate in per-pair SLOTS of two singleton PSUM tiles
  ([128, 2, 128] and [128, 2, 1], one bank each): consecutive pairs' groups
  never overlap in time, so two slots per bank are safe, and the whole PSUM
  budget is 6+1+1 = 8 banks.
- A short burst of dummy matmuls (garbage -> o bank, later overwritten) warms
  the PE p-state during the DMA lead-in so the early score matmuls don't run
  at half clock and starve the exp stream.
- q ships pre-transposed/pre-scaled fp16; pair 0's q column loads first on the
  SP queue ahead of the slabs, the rest rides the Activation queue.
- Slabs stream on the SP queue in 8-chunk [K|V] units (2KB/partition fp8);
  output stores are issued after the slab stream (HWDGE; a prepared-SWDGE
  trigger store would shave the tail but TimelineSim deadlocks on it).

TimelineSim: ~38us/core; ACT ~32us busy (the bottleneck), PE ~28us,
DMA_ENGINES ~26us, DVE ~5us.
"""

from contextlib import ExitStack

import ml_dtypes
import numpy as np

import concourse.bass as bass
import concourse.mybir as mybir
import concourse.tile as tile
from concourse import bacc, bass_utils

# Problem dims (hardcoded per the harness contract)
B, SQ, S_TOTAL = 16, 32, 2048
H, HKV, D = 32, 8, 128
G = H // HKV
SCALE = 0.08838834764831845
N_CORES = 8
B_LOC = B // N_CORES               # 2 sequences per core

P = 128
NPAIR = B_LOC * HKV                # 16 (seq, kv-head) pairs per core
NC_TOT = S_TOTAL // P              # 16 s-chunks per pair
NG = NPAIR * NC_TOT                # 256 chunks in the stream, g = pair*16 + c

# DMA units (g0, ng): [K(g0..g0+ng) | V(g0..g0+ng)] fp8 slabs, 2*ng*128 bytes
# per partition. Small first units so the first scores fire early; 16-chunk
# units mid-stream to halve the serial per-instruction HWDGE cost.
DMA_UNITS = [(0, 8), (8, 8), (16, 8), (24, 8), (32, 8), (40, 8)]
DMA_UNITS += [(48 + 16 * k, 16) for k in range(13)]
assert sum(ng for _, ng in DMA_UNITS) == NG
MAX_NG = max(ng for _, ng in DMA_UNITS)

# q pieces (pair_lo, pair_hi), loaded on demand with one-pair lookahead so the
# 1.4us of q traffic is spread between kv units and never delays the kv chunk
# a pending exp is waiting for.
Q_PIECES = [(0, 1), (1, 2), (2, 4), (4, 8), (8, 16)]

# Score/exp tiles (g0, ntile): ONE whole-tile exp per tile (sub-splitting a
# shared PSUM tile creates false WAR deps between a sub-exp and the next sub's
# score matmuls). Sizes ramp up at the lead (early ACT start) and down at the
# tail (short PV trail); 12-chunk (3-bank) tiles mid-stream.
S_TILES = [(0, 2), (2, 2), (4, 4)]
S_TILES += [(8 + 8 * t, 8) for t in range(30)]
S_TILES += [(248, 4), (252, 2), (254, 2)]
assert sum(nt for _, nt in S_TILES) == NG
TILE_W = 8                         # score tile free width (2 PSUM banks)

N_DUMMY = 22                       # PE p-state warmup matmuls

# Fast-exp offload: some mid-stream exp tiles run on the idle DVE instead of
# ACT via the Schraudolph bit trick in fp16: p = bitcast_fp16(int16(round16(
# A16*s + B16 - C))). Costs ~1.45% rms relative error on the offloaded
# probabilities (calibrated C); trades it for ~1.5us of ACT time per tile.
N_FAST = 5
FAST_C = 55.0
A16 = 1024 * 1.4426950408889634
B16 = float(15 * 1024)
_TF = [i for i, (_, nt) in enumerate(S_TILES) if nt == TILE_W]
_STEP = max(1, len(_TF) // max(N_FAST, 1))
FAST_TILES = {_TF[i * _STEP + 2] for i in range(N_FAST)}

F32 = mybir.dt.float32
F16 = mybir.dt.float16
F8 = mybir.dt.float8e3             # e3m4: 4 mantissa bits
NP_F8 = ml_dtypes.float8_e3m4

_CACHED_NC = {}


def _build_nc():
    nc = bacc.Bacc("TRN2", target_bir_lowering=False, debug=False,
                   enable_asserts=False, num_devices=N_CORES)

    W_TOTAL = 2 * NG * P
    od = nc.dram_tensor("o", [NPAIR // 2, P, 2 * P], F16, kind="ExternalOutput").ap()
    qtd = nc.dram_tensor("qt", [P, NPAIR * P], F16, kind="ExternalInput").ap()
    kvd = nc.dram_tensor("kv", [P, W_TOTAL], F8, kind="ExternalInput").ap()

    with tile.TileContext(nc) as tc, ExitStack() as ctx:
        with (
            tc.tile_pool(name="singles", bufs=1) as singles,
            tc.tile_pool(name="kvp", bufs=6) as kv_pool,
            tc.tile_pool(name="pTp", bufs=4) as pT_pool,
            tc.tile_pool(name="oop", bufs=8) as oo_pool,
            tc.tile_pool(name="smp", bufs=4) as small_pool,
            tc.tile_pool(name="fxp", bufs=2) as fast_pool,
            tc.tile_pool(name="sps", bufs=3, space="PSUM") as s_pool,
            tc.tile_pool(name="ops", bufs=1, space="PSUM") as o_pool,
            tc.tile_pool(name="lps", bufs=1, space="PSUM") as l_pool,
        ):
            ones_col = singles.tile([P, 1], F16)
            nc.vector.memset(ones_col[:], 1.0)
            garbage = singles.tile([P, P], F16)
            nc.vector.memset(garbage[:], 1.0)

            # q^T [d, pair, q] in separate per-piece tiles (hard dep
            # separation), loaded just-in-time between kv units.
            q_tiles = []
            for lo, hi in Q_PIECES:
                qp = singles.tile([P, hi - lo, P], F16, name=f"qp{lo}")
                q_tiles.append((lo, hi, qp))

            q_loaded = set()

            def load_q_piece(piece):
                if piece in q_loaded or piece >= len(q_tiles):
                    return
                q_loaded.add(piece)
                lo, hi, qp = q_tiles[piece]
                nc.sync.dma_start(
                    qp[:], qtd[:, lo * P:hi * P].rearrange(
                        "p (i j) -> p i j", i=hi - lo, j=P))

            def piece_of(i):
                for piece, (lo, hi, _) in enumerate(q_tiles):
                    if lo <= i < hi:
                        return piece
                raise AssertionError(i)

            def q_ap(i):
                load_q_piece(piece_of(i))
                # lookahead: next piece issues two pairs early
                if i + 1 < NPAIR:
                    load_q_piece(piece_of(i + 1))
                lo, hi, qp = q_tiles[piece_of(i)]
                return qp[:, i - lo, :]

            # o/l accumulator slots: one bank each, slot = pair % 2.
            o_ps = o_pool.tile([P, 2, P], F32)
            l_ps = l_pool.tile([P, 2, 1], F32)

            # PE p-state warmup: garbage matmuls into o slot 0 (overwritten by
            # the first real PV accumulation group).
            for _ in range(N_DUMMY):
                nc.tensor.matmul(o_ps[:, 0, :], garbage[:], garbage[:],
                                 start=True, stop=True)

            # DMA stream bookkeeping
            offs = np.cumsum([0] + [2 * ng * P for _, ng in DMA_UNITS])
            du_of_g = {}               # g -> (unit_idx, g0, ng)
            for ui, (g0, ng) in enumerate(DMA_UNITS):
                for g in range(g0, g0 + ng):
                    du_of_g[g] = (ui, g0, ng)
            kv_tiles = {}              # unit_idx -> tile

            def load_unit(ui):
                g0, ng = DMA_UNITS[ui]
                w = 2 * ng * P
                kv_t = kv_pool.tile([P, 2 * MAX_NG * P], F8, tag="kv")
                nc.sync.dma_start(kv_t[:, 0:w],
                                  kvd[:, int(offs[ui]):int(offs[ui]) + w])
                kv_tiles[ui] = kv_t

            LAG = 2                    # PV trails scores by 2 s-tiles
            pend = {}                  # tile idx -> (g0, nt, pT)
            store_q = []               # deferred output stores
            NT = len(S_TILES)
            for t in range(NT + LAG):
                if t < NT:
                    g0, nt = S_TILES[t]
                    sps = s_pool.tile([P, TILE_W, P], F32, tag="sps",
                                      name=f"sps{t}")
                    pT = pT_pool.tile([P, TILE_W, P], F16, tag="pT",
                                      name=f"pT{t}")
                    for pos in range(nt):
                        g = g0 + pos
                        ui, ug0, ung = du_of_g[g]
                        if ui not in kv_tiles:
                            load_unit(ui)
                        kv_t = kv_tiles[ui]
                        kT = kv_t[:, 0:ung * P].rearrange(
                            "p (c s) -> p c s", c=ung, s=P)
                        nc.tensor.matmul(sps[:, pos, :], kT[:, g - ug0, :],
                                         q_ap(g // NC_TOT),
                                         start=True, stop=True)
                    tmp = None
                    if t in FAST_TILES:
                        # pass1 frees the PSUM score slot; pass2 (the int16
                        # bitcast) is deferred to the PV batch two tiles later
                        # so divides can interleave on the DVE queue.
                        tmp = fast_pool.tile([P, TILE_W, P], F16, tag="fexp",
                                             name=f"fexp{t}")
                        nc.vector.tensor_scalar(
                            tmp[:, 0:nt, :], sps[:, 0:nt, :],
                            A16, B16 - FAST_C,
                            mybir.AluOpType.mult, mybir.AluOpType.add)
                    else:
                        nc.scalar.activation(
                            pT[:, 0:nt, :], sps[:, 0:nt, :],
                            mybir.ActivationFunctionType.Exp)
                    pend[t] = (g0, nt, pT, tmp)
                if t >= LAG:
                    g0, nt, pT, tmp = pend.pop(t - LAG)
                    if tmp is not None:
                        nc.vector.tensor_copy(
                            pT[:, 0:nt, :].bitcast(mybir.dt.int16),
                            tmp[:, 0:nt, :])
                    for pos in range(nt):
                        g = g0 + pos
                        i, c = g // NC_TOT, g % NC_TOT
                        ui, ug0, ung = du_of_g[g]
                        kv_t = kv_tiles[ui]
                        vv = kv_t[:, ung * P:2 * ung * P].rearrange(
                            "p (c e) -> p c e", c=ung, e=P)
                        nc.tensor.matmul(o_ps[:, i % 2, :], pT[:, pos, :],
                                         vv[:, g - ug0, :],
                                         start=(c == 0), stop=(c == NC_TOT - 1))
                        nc.tensor.matmul(l_ps[:, i % 2, :], pT[:, pos, :],
                                         ones_col[:],
                                         start=(c == 0), stop=(c == NC_TOT - 1))
                        if c == NC_TOT - 1:
                            linv = small_pool.tile([P, 1], F32, tag="linv")
                            nc.vector.reciprocal(linv[:], l_ps[:, i % 2, :])
                            if i % 2 == 0:
                                oo = oo_pool.tile([P, 2 * P], F16, tag="oo")
                            nc.vector.tensor_scalar_mul(
                                oo[:, (i % 2) * P:(i % 2 + 1) * P],
                                o_ps[:, i % 2, :], linv[:])
                            if i % 2 == 1:
                                store_q.append((i // 2, oo))

            for j, oo_p in store_q:
                nc.sync.dma_start(od[j], oo_p[:])

    nc.compile()
    return nc


def get_nc():
    if "nc" not in _CACHED_NC:
        _CACHED_NC["nc"] = _build_nc()
    return _CACHED_NC["nc"]


def shard_inputs(q, k, v, k_cache, v_cache, slot_mapping):
    """Apply the KV scatter and build per-core slab/qT input maps."""
    k_new = np.asarray(k).reshape(-1, HKV, D)
    v_new = np.asarray(v).reshape(-1, HKV, D)
    sm = np.asarray(slot_mapping)
    kc4 = np.asarray(k_cache).reshape(B, S_TOTAL, HKV, D)
    vc4 = np.asarray(v_cache).reshape(B, S_TOTAL, HKV, D)
    q2 = np.asarray(q)

    in_maps = []
    for ci in range(N_CORES):
        b0 = B_LOC * ci
        kc = kc4[b0:b0 + B_LOC].copy()
        vc = vc4[b0:b0 + B_LOC].copy()
        lo, hi = b0 * S_TOTAL, (b0 + B_LOC) * S_TOTAL
        msk = (sm >= lo) & (sm < hi)
        if msk.any():
            idx = sm[msk] - lo
            kc.reshape(-1, HKV, D)[idx] = k_new[msk]
            vc.reshape(-1, HKV, D)[idx] = v_new[msk]

        # [b, s, hh, d] -> [pair, p, c, d] with s = p*16 + c
        def chunked(a):
            return (a.transpose(0, 2, 1, 3)
                     .reshape(NPAIR, P, NC_TOT, D))
        kh = chunked(kc).astype(NP_F8)
        vh = chunked(vc).astype(NP_F8)
        kT = kh.transpose(0, 3, 2, 1)            # [pair, d, c, p]
        # g-major chunk views, g = pair*16 + c
        kTg = kT.transpose(0, 2, 1, 3).reshape(NG, P, P)   # [g, d, p]
        vg = vh.transpose(0, 2, 1, 3).reshape(NG, P, D)    # [g, p, d]

        parts = []
        for g0, ng in DMA_UNITS:
            parts.append(kTg[g0:g0 + ng].transpose(1, 0, 2).reshape(P, ng * P))
            parts.append(vg[g0:g0 + ng].transpose(1, 0, 2).reshape(P, ng * P))
        kv = np.concatenate(parts, axis=1)

        # q^T: [b, q, hh, g, d] -> [d, pair, q*G+g], pre-scaled
        qc = (q2[b0 * SQ:(b0 + B_LOC) * SQ]
              .reshape(B_LOC, SQ, HKV, G, D)
              .transpose(0, 2, 1, 3, 4)
              .reshape(NPAIR, SQ * G, D)
              .transpose(2, 0, 1)) * SCALE

        in_maps.append({
            "qt": np.ascontiguousarray(
                qc.reshape(P, NPAIR * P).astype(np.float16)),
            "kv": np.ascontiguousarray(kv),
        })
    return in_maps


def _unshard(results):
    outs = []
    for ci in range(N_CORES):
        o_dev = np.asarray(results[ci]["o"], dtype=np.float32)
        o_pair = (o_dev.reshape(NPAIR // 2, P, 2, P)
                  .transpose(0, 2, 1, 3)
                  .reshape(B_LOC, HKV, SQ, G, D)
                  .transpose(0, 2, 1, 3, 4)
                  .reshape(B_LOC * SQ, H * D))
        outs.append(o_pair)
    return np.concatenate(outs, axis=0)


def kernel(q, k, v, k_cache, v_cache, slot_mapping, _trace=False):
    in_maps = shard_inputs(q, k, v, k_cache, v_cache, slot_mapping)
    nc = get_nc()
    res = bass_utils.run_bass_kernel_spmd(
        nc, in_maps, core_ids=list(range(N_CORES)), trace=_trace)
    out = _unshard(res.results)
    if _trace:
        kernel.last_results = res
    return out


# revision 53
# speedup vs baseline: 1.0112x; 1.0112x over previous
"""Paged GQA attention (sparse_attention nn_Attention_29867202576782) on 8 trn2 cores.

Data-parallel over the B=16 sequences (2 per core); 16 (seq, kv-head) pairs per
core, each attending 128 query-columns (SQ*G) over S_TOTAL=2048 keys. K and V
ship as fp8e3 (e3m4) slabs - the PE supports mixed-dtype matmuls (fp8 stationary
x fp16 moving and vice versa), which halves the KV DMA traffic (rel err 1.76e-2
vs the 2e-2 gate, measured end-to-end on HW with the real inputs).

With fp8 the kernel is ACT-bound (one exp pass over all 4.2M scores/core at
1 elem/lane/cycle), so everything is organized around a gapless Activation
stream:

- The 256-chunk (pair, s-chunk) stream is tiled into [128, 12, 128] fp32 PSUM
  score tiles (3 banks x 2 bufs) that deliberately cross pair boundaries: one
  exp instruction per 12-chunk tile amortizes the ~185ns fixed ACT access cost
  (26 exps instead of 33+). The first/last tiles are split into 2/2/4/4-chunk
  exps to start the stream early and retire it cheaply.
- o and l accumulate in per-pair SLOTS of two singleton PSUM tiles
  ([128, 2, 128] and [128, 2, 1], one bank each): consecutive pairs' groups
  never overlap in time, so two slots per bank are safe, and the whole PSUM
  budget is 6+1+1 = 8 banks.
- A short burst of dummy matmuls (garbage -> o bank, later overwritten) warms
  the PE p-state during the DMA lead-in so the early score matmuls don't run
  at half clock and starve the exp stream.
- q ships pre-transposed/pre-scaled fp16; pair 0's q column loads first on the
  SP queue ahead of the slabs, the rest rides the Activation queue.
- Slabs stream on the SP queue in 8-chunk [K|V] units (2KB/partition fp8);
  output stores are issued after the slab stream (HWDGE; a prepared-SWDGE
  trigger store would shave the tail but TimelineSim deadlocks on it).

TimelineSim: ~38us/core; ACT ~32us busy (the bottleneck), PE ~28us,
DMA_ENGINES ~26us, DVE ~5us.
"""

from contextlib import ExitStack

import ml_dtypes
import numpy as np

import concourse.bass as bass
import concourse.mybir as mybir
import concourse.tile as tile
from concourse import bacc, bass_utils

# Problem dims (hardcoded per the harness contract)
B, SQ, S_TOTAL = 16, 32, 2048
H, HKV, D = 32, 8, 128
G = H // HKV
SCALE = 0.08838834764831845
N_CORES = 8
B_LOC = B // N_CORES               # 2 sequences per core

P = 128
NPAIR = B_LOC * HKV                # 16 (seq, kv-head) pairs per core
NC_TOT = S_TOTAL // P              # 16 s-chunks per pair
NG = NPAIR * NC_TOT                # 256 chunks in the stream, g = pair*16 + c

# DMA units (g0, ng): [K(g0..g0+ng) | V(g0..g0+ng)] fp8 slabs, 2*ng*128 bytes
# per partition. Small first units so the first scores fire early; 16-chunk
# units mid-stream to halve the serial per-instruction HWDGE cost.
DMA_UNITS = [(0, 8), (8, 8), (16, 8), (24, 8), (32, 8), (40, 8)]
DMA_UNITS += [(48 + 16 * k, 16) for k in range(13)]
assert sum(ng for _, ng in DMA_UNITS) == NG
MAX_NG = max(ng for _, ng in DMA_UNITS)

# q pieces (pair_lo, pair_hi), loaded on demand with one-pair lookahead so the
# 1.4us of q traffic is spread between kv units and never delays the kv chunk
# a pending exp is waiting for.
Q_PIECES = [(0, 2), (2, 8), (8, 16)]

# Score/exp tiles (g0, ntile): ONE whole-tile exp per tile (sub-splitting a
# shared PSUM tile creates false WAR deps between a sub-exp and the next sub's
# score matmuls). Sizes ramp up at the lead (early ACT start) and down at the
# tail (short PV trail); 12-chunk (3-bank) tiles mid-stream.
S_TILES = [(0, 2), (2, 2), (4, 4)]
S_TILES += [(8 + 8 * t, 8) for t in range(30)]
S_TILES += [(248, 4), (252, 2), (254, 2)]
assert sum(nt for _, nt in S_TILES) == NG
TILE_W = 8                         # score tile free width (2 PSUM banks)

N_DUMMY = 22                       # PE p-state warmup matmuls

# Fast-exp offload: some mid-stream exp tiles run on the idle DVE instead of
# ACT via the Schraudolph bit trick in fp16: p = bitcast_fp16(int16(round16(
# A16*s + B16 - C))). Costs ~1.45% rms relative error on the offloaded
# probabilities (calibrated C); trades it for ~1.5us of ACT time per tile.
N_FAST = 4
FAST_C = 55.0
A16 = 1024 * 1.4426950408889634
B16 = float(15 * 1024)
_TF = [i for i, (_, nt) in enumerate(S_TILES) if nt == TILE_W]
_STEP = max(1, len(_TF) // max(N_FAST, 1))
FAST_TILES = {_TF[i * _STEP + 2] for i in range(N_FAST)}

F32 = mybir.dt.float32
F16 = mybir.dt.float16
F8 = mybir.dt.float8e3             # e3m4: 4 mantissa bits
NP_F8 = ml_dtypes.float8_e3m4

_CACHED_NC = {}


def _build_nc():
    nc = bacc.Bacc("TRN2", target_bir_lowering=False, debug=False,
                   enable_asserts=False, num_devices=N_CORES)

    W_TOTAL = 2 * NG * P
    od = nc.dram_tensor("o", [NPAIR // 2, P, 2 * P], F16, kind="ExternalOutput").ap()
    qtd = nc.dram_tensor("qt", [P, NPAIR * P], F16, kind="ExternalInput").ap()
    kvd = nc.dram_tensor("kv", [P, W_TOTAL], F8, kind="ExternalInput").ap()

    with tile.TileContext(nc) as tc, ExitStack() as ctx:
        with (
            tc.tile_pool(name="singles", bufs=1) as singles,
            tc.tile_pool(name="kvp", bufs=6) as kv_pool,
            tc.tile_pool(name="pTp", bufs=4) as pT_pool,
            tc.tile_pool(name="oop", bufs=8) as oo_pool,
            tc.tile_pool(name="smp", bufs=4) as small_pool,
            tc.tile_pool(name="fxp", bufs=2) as fast_pool,
            tc.tile_pool(name="sps", bufs=3, space="PSUM") as s_pool,
            tc.tile_pool(name="ops", bufs=1, space="PSUM") as o_pool,
            tc.tile_pool(name="lps", bufs=1, space="PSUM") as l_pool,
        ):
            ones_col = singles.tile([P, 1], F16)
            nc.vector.memset(ones_col[:], 1.0)
            garbage = singles.tile([P, P], F16)
            nc.vector.memset(garbage[:], 1.0)

            # q^T [d, pair, q] in separate per-piece tiles (hard dep
            # separation), loaded just-in-time between kv units.
            q_tiles = []
            for lo, hi in Q_PIECES:
                qp = singles.tile([P, hi - lo, P], F16, name=f"qp{lo}")
                q_tiles.append((lo, hi, qp))

            q_loaded = set()

            def load_q_piece(piece):
                if piece in q_loaded or piece >= len(q_tiles):
                    return
                q_loaded.add(piece)
                lo, hi, qp = q_tiles[piece]
                nc.sync.dma_start(
                    qp[:], qtd[:, lo * P:hi * P].rearrange(
                        "p (i j) -> p i j", i=hi - lo, j=P))

            def piece_of(i):
                for piece, (lo, hi, _) in enumerate(q_tiles):
                    if lo <= i < hi:
                        return piece
                raise AssertionError(i)

            def q_ap(i):
                load_q_piece(piece_of(i))
                # lookahead: next piece issues two pairs early
                if i + 1 < NPAIR:
                    load_q_piece(piece_of(i + 1))
                lo, hi, qp = q_tiles[piece_of(i)]
                return qp[:, i - lo, :]

            # o/l accumulator slots: one bank each, slot = pair % 2.
            o_ps = o_pool.tile([P, 2, P], F32)
            l_ps = l_pool.tile([P, 2, 1], F32)

            # PE p-state warmup: garbage matmuls into o slot 0 (overwritten by
            # the first real PV accumulation group).
            for _ in range(N_DUMMY):
                nc.tensor.matmul(o_ps[:, 0, :], garbage[:], garbage[:],
                                 start=True, stop=True)

            # DMA stream bookkeeping
            offs = np.cumsum([0] + [2 * ng * P for _, ng in DMA_UNITS])
            du_of_g = {}               # g -> (unit_idx, g0, ng)
            for ui, (g0, ng) in enumerate(DMA_UNITS):
                for g in range(g0, g0 + ng):
                    du_of_g[g] = (ui, g0, ng)
            kv_tiles = {}              # unit_idx -> tile

            def load_unit(ui):
                g0, ng = DMA_UNITS[ui]
                w = 2 * ng * P
                kv_t = kv_pool.tile([P, 2 * MAX_NG * P], F8, tag="kv")
                nc.sync.dma_start(kv_t[:, 0:w],
                                  kvd[:, int(offs[ui]):int(offs[ui]) + w])
                kv_tiles[ui] = kv_t

            LAG = 2                    # PV trails scores by 2 s-tiles
            pend = {}                  # tile idx -> (g0, nt, pT)
            store_q = []               # deferred output stores
            NT = len(S_TILES)
            for t in range(NT + LAG):
                if t < NT:
                    g0, nt = S_TILES[t]
                    sps = s_pool.tile([P, TILE_W, P], F32, tag="sps",
                                      name=f"sps{t}")
                    pT = pT_pool.tile([P, TILE_W, P], F16, tag="pT",
                                      name=f"pT{t}")
                    for pos in range(nt):
                        g = g0 + pos
                        ui, ug0, ung = du_of_g[g]
                        if ui not in kv_tiles:
                            load_unit(ui)
                        kv_t = kv_tiles[ui]
                        kT = kv_t[:, 0:ung * P].rearrange(
                            "p (c s) -> p c s", c=ung, s=P)
                        nc.tensor.matmul(sps[:, pos, :], kT[:, g - ug0, :],
                                         q_ap(g // NC_TOT),
                                         start=True, stop=True)
                    tmp = None
                    if t in FAST_TILES:
                        # pass1 frees the PSUM score slot; pass2 (the int16
                        # bitcast) is deferred to the PV batch two tiles later
                        # so divides can interleave on the DVE queue.
                        tmp = fast_pool.tile([P, TILE_W, P], F16, tag="fexp",
                                             name=f"fexp{t}")
                        nc.vector.tensor_scalar(
                            tmp[:, 0:nt, :], sps[:, 0:nt, :],
                            A16, B16 - FAST_C,
                            mybir.AluOpType.mult, mybir.AluOpType.add)
                    else:
                        nc.scalar.activation(
                            pT[:, 0:nt, :], sps[:, 0:nt, :],
                            mybir.ActivationFunctionType.Exp)
                    pend[t] = (g0, nt, pT, tmp)
                if t >= LAG:
                    g0, nt, pT, tmp = pend.pop(t - LAG)
                    if tmp is not None:
                        nc.vector.tensor_copy(
                            pT[:, 0:nt, :].bitcast(mybir.dt.int16),
                            tmp[:, 0:nt, :])
                    for pos in range(nt):
                        g = g0 + pos
                        i, c = g // NC_TOT, g % NC_TOT
                        ui, ug0, ung = du_of_g[g]
                        kv_t = kv_tiles[ui]
                        vv = kv_t[:, ung * P:2 * ung * P].rearrange(
                            "p (c e) -> p c e", c=ung, e=P)
                        nc.tensor.matmul(o_ps[:, i % 2, :], pT[:, pos, :],
                                         vv[:, g - ug0, :],
                                         start=(c == 0), stop=(c == NC_TOT - 1))
                        nc.tensor.matmul(l_ps[:, i % 2, :], pT[:, pos, :],
                                         ones_col[:],
                                         start=(c == 0), stop=(c == NC_TOT - 1))
                        if c == NC_TOT - 1:
                            linv = small_pool.tile([P, 1], F32, tag="linv")
                            nc.vector.reciprocal(linv[:], l_ps[:, i % 2, :])
                            if i % 2 == 0:
                                oo = oo_pool.tile([P, 2 * P], F16, tag="oo")
                            nc.vector.tensor_scalar_mul(
                                oo[:, (i % 2) * P:(i % 2 + 1) * P],
                                o_ps[:, i % 2, :], linv[:])
                            if i % 2 == 1:
                                store_q.append((i // 2, oo))

            for j, oo_p in store_q:
                nc.sync.dma_start(od[j], oo_p[:])

    nc.compile()
    return nc


def get_nc():
    if "nc" not in _CACHED_NC:
        _CACHED_NC["nc"] = _build_nc()
    return _CACHED_NC["nc"]


def shard_inputs(q, k, v, k_cache, v_cache, slot_mapping):
    """Apply the KV scatter and build per-core slab/qT input maps."""
    k_new = np.asarray(k).reshape(-1, HKV, D)
    v_new = np.asarray(v).reshape(-1, HKV, D)
    sm = np.asarray(slot_mapping)
    kc4 = np.asarray(k_cache).reshape(B, S_TOTAL, HKV, D)
    vc4 = np.asarray(v_cache).reshape(B, S_TOTAL, HKV, D)
    q2 = np.asarray(q)

    in_maps = []
    for ci in range(N_CORES):
        b0 = B_LOC * ci
        kc = kc4[b0:b0 + B_LOC].copy()
        vc = vc4[b0:b0 + B_LOC].copy()
        lo, hi = b0 * S_TOTAL, (b0 + B_LOC) * S_TOTAL
        msk = (sm >= lo) & (sm < hi)
        if msk.any():
            idx = sm[msk] - lo
            kc.reshape(-1, HKV, D)[idx] = k_new[msk]
            vc.reshape(-1, HKV, D)[idx] = v_new[msk]

        # [b, s, hh, d] -> [pair, p, c, d] with s = p*16 + c
        def chunked(a):
            return (a.transpose(0, 2, 1, 3)
                     .reshape(NPAIR, P, NC_TOT, D))
        kh = chunked(kc).astype(NP_F8)
        vh = chunked(vc).astype(NP_F8)
        kT = kh.transpose(0, 3, 2, 1)            # [pair, d, c, p]
        # g-major chunk views, g = pair*16 + c
        kTg = kT.transpose(0, 2, 1, 3).reshape(NG, P, P)   # [g, d, p]
        vg = vh.transpose(0, 2, 1, 3).reshape(NG, P, D)    # [g, p, d]

        parts = []
        for g0, ng in DMA_UNITS:
            parts.append(kTg[g0:g0 + ng].transpose(1, 0, 2).reshape(P, ng * P))
            parts.append(vg[g0:g0 + ng].transpose(1, 0, 2).reshape(P, ng * P))
        kv = np.concatenate(parts, axis=1)

        # q^T: [b, q, hh, g, d] -> [d, pair, q*G+g], pre-scaled
        qc = (q2[b0 * SQ:(b0 + B_LOC) * SQ]
              .reshape(B_LOC, SQ, HKV, G, D)
              .transpose(0, 2, 1, 3, 4)
              .reshape(NPAIR, SQ * G, D)
              .transpose(2, 0, 1)) * SCALE

        in_maps.append({
            "qt": np.ascontiguousarray(
                qc.reshape(P, NPAIR * P).astype(np.float16)),
            "kv": np.ascontiguousarray(kv),
        })
    return in_maps


def _unshard(results):
    outs = []
    for ci in range(N_CORES):
        o_dev = np.asarray(results[ci]["o"], dtype=np.float32)
        o_pair = (o_dev.reshape(NPAIR // 2, P, 2, P)
                  .transpose(0, 2, 1, 3)
                  .reshape(B_LOC, HKV, SQ, G, D)
                  .transpose(0, 2, 1, 3, 4)
                  .reshape(B_LOC * SQ, H * D))
        outs.append(o_pair)
    return np.concatenate(outs, axis=0)


def kernel(q, k, v, k_cache, v_cache, slot_mapping, _trace=False):
    in_maps = shard_inputs(q, k, v, k_cache, v_cache, slot_mapping)
    nc = get_nc()
    res = bass_utils.run_bass_kernel_spmd(
        nc, in_maps, core_ids=list(range(N_CORES)), trace=_trace)
    out = _unshard(res.results)
    if _trace:
        kernel.last_results = res
    return out
